# revision 2
# baseline (speedup 1.0000x reference)
"""Compensated sparse linear: out = x @ (W + delta_B)^T + b on 8 NeuronCores.

Single GEMM with V = W + delta_B, plus bias.

Pattern (microbenchmark-driven, see mb.py): on this toolchain a bf16 matmul
pays a serialized ~115 ns LDWEIGHTS whenever the stationary changes, and
switching PSUM banks between consecutive matmuls costs ~160 ns; fp32r
self-loads its 4-byte weights inside the matmul stream and sustains
~234 ns/MM in the stationary-switching, bank-stable pattern — the fastest
realizable GEMM inner loop here (478 us/core floor for the 2048 matmuls).

Sharding: 2 token shards x 4 out-feature shards; core = r*4 + c. The V
shard (16.8 MB = 128 KB/partition fp32) is the only operand that fits
resident in SBUF at fp32, so V^T stays resident and x streams (67 MB/rep;
100 MB/rep total DMA = 278 us, under the PE floor).

Per core: group (tb, nh) accumulates psum[128T, 512N] over kt:
  lhsT (stationary) = xt[tb][:, kt, :]  [128K, 128T]   (streamed, 2 MB tiles)
  rhs  (moving)     = vt[nh][:, kt, :]  [128K, 512N]   (resident half)
VectorE adds bias during PSUM->SBUF; output in natural [token, feature].

Schedule: the 47 us V^T load is covered by PIN=3 pinned token blocks
(phases A/B, 41 us of PE work); x tiles for the first streamed blocks are
emitted BEFORE the second V half so their DMA is not queued behind it
(the staged baseline stalled 7 us there); the last 3 token blocks run
nh0-then-nh1 (phases D/E) so V half0's pool slot frees ~20 us before the
rep ends, letting the next rep's V reload overlap tail compute. The first
pinned tile and V half are DMA'd per-kt so the first matmul starts ~1 us in.
"""

import numpy as np

import concourse.tile as tile
from concourse import bacc, mybir
from concourse.bass_utils import run_bass_kernel_spmd

P = 128
B, S, D_IN, D_OUT = 4, 2048, 4096, 4096
T = B * S
TR, NCOLS = 2, 4            # token shards x feature shards
T_C, N_C = T // TR, D_OUT // NCOLS
K = D_IN
TB = 128                    # tokens per block (psum partition dim)
NF = 512                    # moving free dim (one PSUM bank, fp32)
KT = K // P                 # 32
TBN = T_C // TB             # 32
NH = N_C // NF              # 2
PIN = 3                     # pinned token blocks covering the V^T load
TAIL = 3                    # trailing blocks split nh0/nh1 for V-slot release

F32R = mybir.dt.float32r
F32 = mybir.dt.float32


def build_nc(reps=1):
    nc = bacc.Bacc("TRN2", target_bir_lowering=False, debug=False, num_devices=8)
    xt_d = nc.dram_tensor("xt", [TBN, P, KT, TB], F32R, kind="ExternalInput").ap()
    vt_d = nc.dram_tensor("vt", [P, KT, N_C], F32R, kind="ExternalInput").ap()
    b_d = nc.dram_tensor("bias", [P, N_C], F32, kind="ExternalInput").ap()
    out_d = nc.dram_tensor("out", [T_C, N_C], F32, kind="ExternalOutput").ap()

    with tile.TileContext(nc) as tc:
        with (
            tc.tile_pool(name="xt", bufs=PIN + 1) as xt_pool,
            tc.tile_pool(name="bias", bufs=1) as b_pool,
            tc.tile_pool(name="outp", bufs=3) as out_pool,
            tc.tile_pool(name="psum", bufs=6, space="PSUM") as psum_pool,
        ):
            with tc.tile_pool(name="vt", bufs=1) as vt_pool:
                bias_s = b_pool.tile([P, N_C], F32)

                def mm_group(xt_s, vt_s, tb, nh):
                    ps = psum_pool.tile([P, NF], F32, name="ps")
                    for kt in range(KT):
                        nc.tensor.matmul(
                            ps[:], xt_s[:, kt, :], vt_s[:, kt, :],
                            start=(kt == 0), stop=(kt == KT - 1),
                        )
                    ot = out_pool.tile([P, NF], F32, name="ot")
                    nc.vector.tensor_add(
                        ot[:], ps[:], bias_s[:, nh * NF:(nh + 1) * NF]
                    )
                    nc.sync.dma_start(
                        out_d[tb * TB:(tb + 1) * TB, nh * NF:(nh + 1) * NF],
                        ot[:],
                    )

                first = True
                for _ in range(reps):
                    pins = {}
                    # first pinned x tile and V half0 stream per-kt,
                    # interleaved, so the first matmul starts ~1us in
                    xt_s = xt_pool.tile([P, KT, TB], F32R, name="xt")
                    vt_halves = {}
                    vt_s = vt_pool.tile([P, KT, NF], F32R, name="vt0")
                    for kt in range(KT):
                        nc.sync.dma_start(xt_s[:, kt, :], xt_d[0, :, kt, :])
                        nc.sync.dma_start(vt_s[:, kt, :], vt_d[:, kt, 0:NF])
                    pins[0] = xt_s
                    vt_halves[0] = vt_s
                    if first:
                        nc.sync.dma_start(bias_s[:], b_d[:])
                        first = False
                    for tb in range(1, PIN):
                        xt_s = xt_pool.tile([P, KT, TB], F32R, name="xt")
                        nc.sync.dma_start(xt_s[:], xt_d[tb])
                        pins[tb] = xt_s
                    # first streamed tile BEFORE V half1 so its DMA is not
                    # stuck behind 16.8 MB of V
                    xt_next = xt_pool.tile([P, KT, TB], F32R, name="xt")
                    nc.sync.dma_start(xt_next[:], xt_d[PIN])

                    vt_s = vt_pool.tile([P, KT, NF], F32R, name="vt1")
                    for kt in range(KT):
                        nc.sync.dma_start(vt_s[:, kt, :], vt_d[:, kt, NF:2 * NF])
                    vt_halves[1] = vt_s

                    for tb in range(PIN):          # phase A
                        mm_group(pins[tb], vt_halves[0], tb, 0)
                    # one streamed nh0 group before phase B: buys the next
                    # rep's V-half1 reload (which can only start at the rep
                    # boundary) time to land before B needs it
                    mm_group(xt_next, vt_halves[0], PIN, 0)
                    for tb in range(PIN):          # phase B
                        mm_group(pins[tb], vt_halves[1], tb, 1)
                    mm_group(xt_next, vt_halves[1], PIN, 1)

                    for tb in range(PIN + 1, TBN - TAIL):   # phase C
                        xt_s = xt_pool.tile([P, KT, TB], F32R, name="xt")
                        nc.sync.dma_start(xt_s[:], xt_d[tb])
                        for nh in range(NH):
                            mm_group(xt_s, vt_halves[nh], tb, nh)

                    tail_tiles = {}
                    for tb in range(TBN - TAIL, TBN):
                        xt_s = xt_pool.tile([P, KT, TB], F32R, name="xt")
                        nc.sync.dma_start(xt_s[:], xt_d[tb])
                        tail_tiles[tb] = xt_s
                    for tb in sorted(tail_tiles):       # phase D (frees half0)
                        mm_group(tail_tiles[tb], vt_halves[0], tb, 0)
                    for tb in sorted(tail_tiles):       # phase E
                        mm_group(tail_tiles[tb], vt_halves[1], tb, 1)
    nc.compile()
    return nc


def shard_layout():
    return [(r, c) for r in range(TR) for c in range(NCOLS)]


def prepare_in_maps(x, W, b, delta_B):
    x2d = np.asarray(x, np.float32).reshape(T, D_IN)
    V = np.asarray(W, np.float32) + np.asarray(delta_B, np.float32)
    b = np.asarray(b, np.float32)

    in_maps = []
    for r, c in shard_layout():
        xs = x2d[r * T_C:(r + 1) * T_C]
        xt = np.ascontiguousarray(xs.reshape(TBN, TB, KT, P).transpose(0, 3, 2, 1))
        Vc = V[c * N_C:(c + 1) * N_C]
        vt = np.ascontiguousarray(Vc.reshape(N_C, KT, P).transpose(2, 1, 0))
        bias = np.ascontiguousarray(np.broadcast_to(b[c * N_C:(c + 1) * N_C], (P, N_C)))
        in_maps.append({"xt": xt, "vt": vt, "bias": bias})
    return in_maps


def assemble_output(results):
    out = np.empty((T, D_OUT), np.float32)
    for i, (r, c) in enumerate(shard_layout()):
        out[r * T_C:(r + 1) * T_C, c * N_C:(c + 1) * N_C] = results[i]["out"]
    return out.reshape(B, S, D_OUT)


def kernel(x, W, b, delta_B):
    nc = build_nc()
    in_maps = prepare_in_maps(x, W, b, delta_B)
    res = run_bass_kernel_spmd(nc, in_maps, list(range(8)))
    return assemble_output(res.results)
